# revision 54
# baseline (speedup 1.0000x reference)
"""ViTDet-style windowed attention w/ decomposed rel-pos, on 8 TRN2 NeuronCores.

Problem: x(8,32,32,768), 12 heads, hd=64, N=1024 tokens per image.
Sharding: pure data-parallel over B - core b handles image b; weights/tables
replicated; no collectives.

Per-core math (all matmuls bf16, fp32 PSUM accumulation):
  qkv^T[o, n]   = qkv_wT.T-chunks @ x^T          (o on partitions, n free)
  scores^T[j,i] = K'^T.T @ Q'^T   per head, where the 128-deep contraction is
                  [k(64) | onehot_jh(32) | onehot_jw(32)] x
                  [q_s(64) | rel_h^T(32) | rel_w^T(32)]
                  -> q.k + rel_h[i,jh] + rel_w[i,jw] in ONE matmul
  E = exp(scores^T)               (no max-subtraction: |scores| < 3)
  [out^T; rowsum] = [v | 1].T @ E (ones column gives softmax denominator free)
  attn_pair[t] = out^T * (1/rowsum), head pair t packed on 128 partitions
  final^T = proj_wT.T @ attn_pair + b_eff  (K=128 per accumulation pass)

Key devices (vs the v0 baseline, TimelineSim 211570 -> 177289 ns,
PE busy 177.3 -> 148.5 us; Act 141.3 us co-critical):
 - Rel-pos phase halved: block-diagonal host-packed tables rhbd/rwbd
   [128, 32, 64] multiply a pair-packed Qrel tile, computing even+odd
   parity rel rows in one 192-col matmul (12288 PE cols vs 24576).
 - Proj contraction K=64 -> K=128: DVE/Act ops can shift partition bases
   (verified on HW), so odd heads' normalized AV output is written to
   partitions 64:128 of per-pair attn_pair tiles; wproj host-packed
   [128, 6, 768] (36864 PE cols vs 73728).
 - 1/rowsum partition-broadcast moved off PE onto Pool
   (gpsimd.partition_broadcast; reciprocal row shifted to partition 0 by
   DVE) - the K=1 f32r broadcast matmuls are gone (-12288 PE cols).
 - Whole-tile dependency tracking => tensors split by independent
   write/read unit (wq0/wqr/wk/wv, QT_e/QT_o, KT_e/KT_o, per-jt Vb and E
   tiles, per-pair attn_pair), so writer chains parallelize across
   engines and readers wait only true producers.
 - Emission order pipelines exp (Act is the second rail at ~100us total):
   q (first 3 tiles sweep column-quarters in DMA-arrival lockstep) ->
   rel(h/w interleaved, v-chunk fillers) -> k6/k7 -> scores0/1 (more
   v fillers) -> attention pipeline -> proj. Engine assignment:
   Act=Qrel/QT_e/rel-even/ALL v-copies/osb-odd (v-copies on Act
   unblocked DVE's front backlog: -9us), DVE=QT_o/KT/rel-odd/recip/
   mul/osb-even, Pool=partition_broadcast.
 - Act runs exp hand-to-mouth behind scores psums (3-slot pool caps
   run-ahead), so ALL non-scores PE work inside the attention phase is
   sliced into <=0.9us chunks (av quarter-chunks, k-tile ct-triples)
   drained from a unified queue ONE PER jt between scores matmuls --
   coarse per-head av/k blocks starved Act ~3us per cycle (-11us).
   Act's stream (12.6->157us) IS the kernel length: in-stream filler
   v-copies are split half-Act/half-DVE to minimize the added stream
   length while keeping the pv psum drain off DVE's critical front.
 - Startup: few big contiguous DMAs ordered by first use across the two
   hwdge queues (DMA issue cost scales with DRAM-side descriptor count).

Constraints discovered (the real-HW verifier enforces): GPSIMD cannot
access PSUM; ISA mem patterns max 3 free dims; activation bias APs only
for non-Copy funcs; exp exists only on the Act engine.
"""

import numpy as np
import ml_dtypes

bf16 = ml_dtypes.bfloat16

B, H, W, C = 8, 32, 32, 768
NH, HD = 12, 64
N = H * W  # 1024
SCALE = HD ** -0.5

_NC = None


def _build(repeat=1):
    import concourse.mybir as mybir
    import concourse.tile as tile
    from concourse import bacc

    BF = mybir.dt.bfloat16
    F32 = mybir.dt.float32
    AF = mybir.ActivationFunctionType
    OP = mybir.AluOpType

    nc = bacc.Bacc(None, target_bir_lowering=False)

    xT_d = nc.declare_dram_parameter("xT", [128, 6, 1024], BF, isOutput=False)
    wqkv_d = nc.declare_dram_parameter("wqkv", [128, 6, 2304], BF, isOutput=False)
    wproj_d = nc.declare_dram_parameter("wproj", [128, 6, 768], BF, isOutput=False)
    bias3_d = nc.declare_dram_parameter("bias3", [128, 24], F32, isOutput=False)
    wq0_d = nc.declare_dram_parameter("wq0", [128, 6, 128], BF, isOutput=False)
    rhbd_d = nc.declare_dram_parameter("rhbd", [128, 32, 64], BF, isOutput=False)
    rwbd_d = nc.declare_dram_parameter("rwbd", [128, 32, 64], BF, isOutput=False)
    eyeh_d = nc.declare_dram_parameter("eyeh", [32, 6, 1024], BF, isOutput=False)
    eyew_d = nc.declare_dram_parameter("eyew", [32, 6, 1024], BF, isOutput=False)
    out_d = nc.declare_dram_parameter("out", [768, 1024], F32, isOutput=True)

    with tile.TileContext(nc) as tc, \
            tc.tile_pool(name="consts", bufs=1) as consts, \
            tc.tile_pool(name="epool", bufs=3) as epool, \
            tc.tile_pool(name="ps", bufs=3, space="PSUM") as pspool, \
            tc.tile_pool(name="pa", bufs=2, space="PSUM") as papool, \
            tc.tile_pool(name="divp", bufs=2) as divp, \
            tc.tile_pool(name="outp", bufs=4) as outp:

        def emit():
            # Whole-tile dependency tracking => split tensors by independent
            # write/read units so writer chains parallelize across engines and
            # readers wait only their true producers.
            xT = consts.tile([128, 6, 1024], BF)
            wq0 = consts.tile([128, 6, 128], BF)     # q o-tile 0 lhsT
            wqr = consts.tile([128, 6, 640], BF)     # q o-tiles 1..5
            wk = consts.tile([128, 6, 768], BF)
            wv = consts.tile([128, 6, 768], BF)
            bias3 = consts.tile([128, 24], F32)
            bqk = bias3[:, 0:12]
            bqkS = bias3[:, 12:18]
            beff = bias3[:, 18:24]
            rhbd = consts.tile([128, 32, 64], BF)
            rwbd = consts.tile([128, 32, 64], BF)
            # startup-critical: few big DMAs; sync and act queues in parallel
            # (each dma_start costs ~1.26us of issue time on its queue).
            nc.sync.dma_start(out=wq0, in_=wq0_d[:])
            nc.scalar.dma_start(out=xT[:, :, 0:256], in_=xT_d[:, :, 0:256])
            nc.sync.dma_start(out=bias3, in_=bias3_d[:])
            nc.sync.dma_start(out=wqr, in_=wqkv_d[:, :, 128:768])
            nc.scalar.dma_start(out=xT[:, :, 256:512], in_=xT_d[:, :, 256:512])
            nc.scalar.dma_start(out=xT[:, :, 512:1024],
                                in_=xT_d[:, :, 512:1024])
            nc.sync.dma_start(out=wv, in_=wqkv_d[:, :, 1536:2304])
            nc.sync.dma_start(out=wk, in_=wqkv_d[:, :, 768:1536])
            nc.scalar.dma_start(out=rhbd, in_=rhbd_d[:])
            nc.scalar.dma_start(out=rwbd, in_=rwbd_d[:])
            # DVE "touch" of DMA-loaded scalars: absorbs the DMA-lane
            # semaphore waits early.
            tch = consts.tile([1, 24], F32)
            nc.vector.tensor_copy(out=tch, in_=bias3[0:1, :])

            # parity-split augmented K'/Q': even heads (q/k rows 0:64, rel
            # 64:128) in _e tiles, odd heads (q/k 64:128, rel 0:64) in _o.
            KT_e = consts.tile([128, 6, 1024], BF)
            KT_o = consts.tile([128, 6, 1024], BF)
            QT_e = consts.tile([128, 6, 1024], BF)
            QT_o = consts.tile([128, 6, 1024], BF)
            Vb = [consts.tile([128, 12, 65], BF, name=f"Vb{j}")
                  for j in range(8)]  # per j-tile: [n-part, head, v|1]
            wproj = consts.tile([128, 6, 768], BF)
            # Qrel (pair-packed scaled q, rel-phase input) dies before
            # attn_pair is born -> share slots.
            Qrel = consts.tile([128, 6, 1024], BF, tag="qr_ap")
            nc.sync.dma_start(out=KT_e[64:96], in_=eyeh_d[:])
            nc.sync.dma_start(out=KT_o[0:32], in_=eyeh_d[:])
            nc.scalar.dma_start(out=KT_e[96:128], in_=eyew_d[:])
            nc.scalar.dma_start(out=KT_o[32:64], in_=eyew_d[:])
            nc.sync.dma_start(out=wproj, in_=wproj_d[:])

            for j in range(8):
                nc.vector.memset(Vb[j][:, :, 64:65], 1.0)

            # ---- Phase 1a: q^T; epilogue -> parity QT + pair-packed Qrel ----
            def qk_tile(ot):
                ps = pspool.tile([128, 1024], F32, tag="ps")
                qk_mms(ps, ot, slice(0, 512))
                qk_mms(ps, ot, slice(512, 1024))
                return ps

            def q_epi(ot, ps):
                nc.scalar.activation(
                    out=Qrel[:, ot, :], in_=ps, func=AF.Identity,
                    bias=bqkS[:, ot:ot + 1], scale=SCALE)
                nc.scalar.activation(
                    out=QT_e[0:64, ot, :], in_=ps[0:64, :], func=AF.Identity,
                    bias=bqkS[0:64, ot:ot + 1], scale=SCALE)
                nc.vector.tensor_scalar(
                    out=QT_o[64:128, ot, :], in0=ps[64:128, :],
                    scalar1=bqk[64:128, ot:ot + 1], scalar2=SCALE,
                    op0=OP.add, op1=OP.mult)

            def qk_mms(ps, ot, csl):
                if ot == 0:
                    wt, sl = wq0, slice(0, 128)
                elif ot < 6:
                    wt, sl = wqr, slice((ot - 1) * 128, ot * 128)
                else:
                    wt, sl = wk, slice((ot - 6) * 128, (ot - 5) * 128)
                for ct in range(6):
                    nc.tensor.matmul(
                        ps[:, csl],
                        wt[:, ct, sl],
                        xT[:, ct, csl],
                        start=(ct == 0), stop=(ct == 5),
                    )

            # first three q tiles run ic=0 passes back-to-back so PE is not
            # gated on the second half of the xT load.
            # first three q tiles sweep column-quarters in lockstep with
            # the staggered xT/wqr DMA arrivals so PE starts at ~2us.
            qtiles = {}
            for ot in range(3):
                qtiles[ot] = pspool.tile([128, 1024], F32, tag="ps",
                                         name=f"qps{ot}")
            for csl in (slice(0, 256), slice(256, 512), slice(512, 1024)):
                for ot in range(3):
                    qk_mms(qtiles[ot], ot, csl)
            for ot in range(3):
                q_epi(ot, qtiles.pop(ot))
            for ot in range(3, 6):
                ps = qk_tile(ot)
                q_epi(ot, ps)

            # ---- Phase 2: rel_h^T / rel_w^T, both parities per matmul -------
            # psum tile packs 8 ii: (rowgrp 2) x (strip: cols 0:192,192:384,
            # 512:704,704:896). out rows 0:32 = even rel(ii), 32:64 = odd.
            def rel_copies(pr, g, table):
                # ISA mem patterns allow <=3 free dims -> one op per
                # (rowgrp, parity, bank): [32p, s2, h6, w32] (384 free).
                for rg in range(2):
                    for b in range(2):
                        src_b = pr[rg * 64:rg * 64 + 64,
                                   b * 512:b * 512 + 384].rearrange(
                            "p (s hw) -> p s hw", s=2).rearrange(
                            "p s (h w) -> p s h w", w=32)
                        c0 = g * 256 + rg * 128 + b * 64
                        if table == "h":
                            de = QT_e[64:96, :, c0:c0 + 64].rearrange(
                                "p h (s w) -> p s h w", s=2)
                            do = QT_o[0:32, :, c0:c0 + 64].rearrange(
                                "p h (s w) -> p s h w", s=2)
                        else:
                            iw0 = g * 8 + rg * 4 + b * 2
                            de = QT_e[96:128].rearrange(
                                "p h (a c) -> p c h a", c=32)[:, iw0:iw0 + 2] \
                                .rearrange("p c h a -> p c h a")
                            do = QT_o[32:64].rearrange(
                                "p h (a c) -> p c h a", c=32)[:, iw0:iw0 + 2] \
                                .rearrange("p c h a -> p c h a")
                        nc.scalar.activation(out=de, in_=src_b[0:32],
                                             func=AF.Identity, bias=0.0,
                                             scale=1.0)
                        nc.vector.tensor_copy(out=do, in_=src_b[32:64])

            Qrel_w = Qrel.rearrange("p h (a b) -> p h a b", b=32)

            def rel_group(g, table):
                pr = pspool.tile([128, 1024], F32, tag="ps")
                for k in range(8):
                    ii = g * 8 + k
                    rg, s = k // 4, k % 4
                    col = (s // 2) * 512 + (s % 2) * 192
                    if table == "h":
                        rhs = Qrel[:, :, ii * 32:(ii + 1) * 32]
                        tbl = rhbd
                    else:
                        rhs = Qrel_w[:, :, :, ii]
                        tbl = rwbd
                    nc.tensor.matmul(
                        pr[rg * 64:(rg + 1) * 64, col:col + 192],
                        tbl[:, ii, :], rhs,
                        start=True, stop=True, tile_position=(0, rg * 64))
                rel_copies(pr, g, table)

            def k_tile(ot, on_act=False):
                ps = qk_tile(ot)
                t = ot - 6
                if on_act:
                    nc.scalar.activation(
                        out=KT_e[0:64, t, :], in_=ps[0:64, :],
                        func=AF.Identity, bias=bqk[0:64, ot:ot + 1], scale=1.0)
                    nc.scalar.activation(
                        out=KT_o[64:128, t, :], in_=ps[64:128, :],
                        func=AF.Identity, bias=bqk[64:128, ot:ot + 1],
                        scale=1.0)
                else:
                    nc.vector.tensor_scalar(
                        out=KT_e[0:64, t, :], in0=ps[0:64, :],
                        scalar1=bqk[0:64, ot:ot + 1], scalar2=None, op0=OP.add)
                    nc.vector.tensor_scalar(
                        out=KT_o[64:128, t, :], in0=ps[64:128, :],
                        scalar1=bqk[64:128, ot:ot + 1], scalar2=None,
                        op0=OP.add)

            # h/w groups interleaved so the QT_e (Act) and QT_o (DVE)
            # write chains stay data-paced; k6 + first v chunks fill PE while
            # the copy engines drain rel psum tiles.
            rel_filler = []

            def rel_phase():
                rel_group(0, "h")
                rel_group(0, "w")
                rel_group(1, "h")
                rel_group(1, "w")
                rel_filler.pop(0)()
                rel_group(2, "h")
                rel_group(2, "w")
                rel_filler.pop(0)()
                rel_group(3, "h")
                rel_group(3, "w")
                while rel_filler:
                    rel_filler.pop(0)()

            # ---- attn pipeline: scores+exp / [v|1]@E; v before any AV ------
            fillers = []
            kfillers = []

            def push_k_fillers(ot):
                # k-tile split into two 6-mm halves emitted at jt6/jt7 of the
                # preceding scores head: the mms overlap Act's exp backlog
                # instead of starving it, and the KT epilogues land after all
                # of that head's KT reads but before the next pair's.
                st = {}

                def kmms(ps, csl, cts):
                    wsl = slice((ot - 6) * 128, (ot - 5) * 128)
                    for ct in cts:
                        nc.tensor.matmul(ps[:, csl], wk[:, ct, wsl],
                                         xT[:, ct, csl],
                                         start=(ct == 0), stop=(ct == 5))

                def c1():
                    st["ps"] = pspool.tile([128, 1024], F32, tag="ps",
                                           name=f"kps{ot}")
                    kmms(st["ps"], slice(0, 512), range(3))

                def c2():
                    kmms(st["ps"], slice(0, 512), range(3, 6))

                def c3():
                    kmms(st["ps"], slice(512, 1024), range(3))

                def c4():
                    ps = st.pop("ps")
                    kmms(ps, slice(512, 1024), range(3, 6))
                    t = ot - 6
                    nc.vector.tensor_scalar(
                        out=KT_e[0:64, t, :], in0=ps[0:64, :],
                        scalar1=bqk[0:64, ot:ot + 1], scalar2=None, op0=OP.add)
                    nc.vector.tensor_scalar(
                        out=KT_o[64:128, t, :], in0=ps[64:128, :],
                        scalar1=bqk[64:128, ot:ot + 1], scalar2=None,
                        op0=OP.add)

                kfillers.extend([c1, c2, c3, c4])

            def head_scores(h):
                KT = KT_e if h % 2 == 0 else KT_o
                QT = QT_e if h % 2 == 0 else QT_o
                t = h // 2
                E = [epool.tile([128, 1024], BF, tag=f"E{jt}",
                                name=f"E{h}_{jt}") for jt in range(8)]
                for jt in range(8):
                    ps = pspool.tile([128, 1024], F32, tag="ps")
                    for ic in range(2):
                        nc.tensor.matmul(ps[:, ic * 512:(ic + 1) * 512],
                                         KT[:, t, jt * 128:(jt + 1) * 128],
                                         QT[:, t, ic * 512:(ic + 1) * 512],
                                         start=True, stop=True)
                    nc.scalar.activation(out=E[jt], in_=ps, func=AF.Exp)
                    if avq:
                        avq.pop(0)()
                    elif kfillers:
                        kfillers.pop(0)()
                    elif fillers and (h >= 2 or jt % 2 == 1):
                        fillers.pop(0)()
                return E

            attn_pair = [consts.tile([128, 1024], BF, tag=f"ap{t}",
                                     name=f"attn_pair{t}")
                         for t in range(6)]

            avq = []

            def av_mms(pa, h, E, ic, jts):
                for jt in jts:
                    nc.tensor.matmul(pa[0:65, :], Vb[jt][:, h, 0:65],
                                     E[jt][:, ic * 512:(ic + 1) * 512],
                                     start=(jt == 0), stop=(jt == 7))

            def av_epi(pa, h, ic):
                t, par = divmod(h, 2)
                rec = divp.tile([1, 512], F32, tag="rec")
                with nc.allow_low_precision(reason="dve recip approx ok"):
                    nc.vector.reciprocal(rec, pa[64:65, :])
                bc = divp.tile([64, 512], F32, tag="bc")
                nc.gpsimd.partition_broadcast(bc, rec[0:1, :])
                dst = attn_pair[t][par * 64:par * 64 + 64,
                                   ic * 512:(ic + 1) * 512]
                nc.vector.tensor_mul(dst, pa[0:64, :], bc)

            def av_half(h, E, ic):
                pa = papool.tile([65, 512], F32, tag="pa")
                av_mms(pa, h, E, ic, range(8))
                av_epi(pa, h, ic)

            def head_av(h, E):
                av_half(h, E, 0)
                av_half(h, E, 1)

            def push_av(h, E):
                # quarter-chunks (~0.85us) so Act's exp appetite (1.04us/jt)
                # is never starved by a long PE filler block
                for ic in range(2):
                    st = {}

                    def q1(ic=ic, st=st):
                        st["pa"] = papool.tile([65, 512], F32, tag="pa",
                                               name=f"pa{h}_{ic}")
                        av_mms(st["pa"], h, E, ic, range(4))

                    def q2(ic=ic, st=st):
                        pa = st.pop("pa")
                        av_mms(pa, h, E, ic, range(4, 8))
                        av_epi(pa, h, ic)

                    avq.append(q1)
                    avq.append(q2)

            def v_chunk(nt, ovc, on_act=False, split=False):
                pv = papool.tile([128, 384], F32, tag="pa")
                for ct in range(6):
                    nc.tensor.matmul(
                        pv,
                        xT[:, ct, nt * 128:(nt + 1) * 128],
                        wv[:, ct, ovc * 384:(ovc + 1) * 384],
                        start=(ct == 0), stop=(ct == 5),
                    )
                src = pv.rearrange("p (h d) -> p h d", d=64)
                dst = Vb[nt][:, ovc * 6:(ovc + 1) * 6, 0:64]
                if split:
                    # halve the in-exp-stream Act burden: Act and DVE each
                    # drain three heads' worth of this pv tile
                    nc.scalar.activation(out=dst[:, 0:3], in_=src[:, 0:3],
                                         func=AF.Identity, bias=0.0, scale=1.0)
                    nc.vector.tensor_copy(out=dst[:, 3:6], in_=src[:, 3:6])
                elif on_act:
                    nc.scalar.activation(out=dst, in_=src, func=AF.Identity,
                                         bias=0.0, scale=1.0)
                else:
                    nc.vector.tensor_copy(out=dst, in_=src)

            # k(6+t) emitted just before scores(2t) so each scores pair
            # waits only its own k epilogue in the KT chain. All 16 v chunks
            # complete before av(0): 7 up front (copies on Pool), 9 as
            # PE-fillers inside scores(0)/(1) (copies on DVE).
            chunks = [(nt, ovc) for nt in range(8) for ovc in range(2)]
            # v chunks fill the PE gap while the Act Qrel chain finishes;
            # k6/k7 before rel so their DVE epilogues precede the rel-copy
            # backlog and scores(0..3) aren't gated on it.
            for nt, ovc in chunks[:5]:
                v_chunk(nt, ovc, on_act=True)
            k_tile(6)
            k_tile(7)
            rel_filler += [
                (lambda a, b: lambda: v_chunk(a, b, on_act=True))(nt, ovc)
                for nt, ovc in chunks[5:7]
            ]
            rel_phase()
            # filler v-copies go to DVE: they pop inside scores(0)/(1),
            # i.e. INSIDE Act's saturated exp stream, while DVE has slack
            # there now that av work is fine-grained (av0 starts ~15us later).
            fillers += [
                (lambda a, b: lambda: v_chunk(a, b, split=True))(nt, ovc)
                for nt, ovc in chunks[7:]
            ]
            Es = {0: head_scores(0), 1: head_scores(1)}
            for h in range(2, 12):
                if h % 2 == 1 and h <= 9:
                    push_k_fillers(6 + (h + 1) // 2)
                Es[h] = head_scores(h)
                head_av(h - 2, Es.pop(h - 2))
            # ---- Phase 5: proj (K=128 over head pairs) + b_eff -------------
            # Two 3-cot groups; pairs 0..4 accumulate first so group A's bulk
            # fills PE while av(10)/av(11) wait on the last exps; only the
            # pair-5 passes and group B trail av(11).
            def proj_mm(ps, cot, t):
                for ic in range(2):
                    nc.tensor.matmul(ps[:, ic * 512:(ic + 1) * 512],
                                     wproj[:, t, cot * 128:(cot + 1) * 128],
                                     attn_pair[t][:, ic * 512:(ic + 1) * 512],
                                     start=(t == 0), stop=(t == 5))

            def proj_epi(ps, cot):
                for ic in range(2):
                    osb = outp.tile([128, 512], F32, tag="osb")
                    if ic == 0:
                        nc.vector.tensor_scalar(
                            out=osb, in0=ps[:, 0:512],
                            scalar1=beff[:, cot:cot + 1], scalar2=None,
                            op0=OP.add)
                    else:
                        nc.scalar.activation(
                            out=osb, in_=ps[:, 512:1024], func=AF.Identity,
                            bias=beff[:, cot:cot + 1], scale=1.0)
                    eng = nc.sync if ic == 0 else nc.scalar
                    eng.dma_start(
                        out=out_d[cot * 128:(cot + 1) * 128,
                                  ic * 512:(ic + 1) * 512],
                        in_=osb)

            head_av(10, Es.pop(10))
            head_av(11, Es.pop(11))
            for cot in range(6):
                ps = pspool.tile([128, 1024], F32, tag="ps")
                for t in range(6):
                    proj_mm(ps, cot, t)
                proj_epi(ps, cot)

        for _rep in range(repeat):
            emit()

    nc.compile()
    return nc


def _get_nc():
    global _NC
    if _NC is None:
        _NC = _build()
    return _NC


def _prep_inputs(x, qkv_w, qkv_b, proj_w, proj_b, rel_pos_h, rel_pos_w):
    x = np.asarray(x, np.float32)
    qkv_w = np.asarray(qkv_w, np.float32)
    qkv_b = np.asarray(qkv_b, np.float32)
    proj_w = np.asarray(proj_w, np.float32)
    proj_b = np.asarray(proj_b, np.float32)
    rel_pos_h = np.asarray(rel_pos_h, np.float32)
    rel_pos_w = np.asarray(rel_pos_w, np.float32)

    wqkv = np.ascontiguousarray(
        qkv_w.T.reshape(6, 128, 3 * C).transpose(1, 0, 2)).astype(bf16)
    wproj = np.ascontiguousarray(
        proj_w.T.reshape(6, 128, C).transpose(1, 0, 2)).astype(bf16)
    bqk = qkv_b[:2 * C].reshape(12, 128).T.astype(np.float32)
    # Act Identity epilogue computes in*SCALE + bias -> bias pre-scaled
    bqkS = bqk[:, 0:6] * SCALE
    beff = (proj_w @ qkv_b[2 * C:] + proj_b).reshape(6, 128).T.astype(np.float32)
    bias3 = np.ascontiguousarray(
        np.concatenate([bqk, bqkS, beff], axis=1)).astype(np.float32)
    wq0 = np.ascontiguousarray(
        qkv_w.T.reshape(6, 128, 3 * C).transpose(1, 0, 2)[:, :, 0:128]
    ).astype(bf16)

    coords = np.arange(32)[:, None] - np.arange(32)[None, :] + 31
    # [c, ih, jh] tables -> block-diag [128, 32, 64]: rows 0:64 cols 0:32 for
    # even-parity q, rows 64:128 cols 32:64 for odd-parity q.
    rhT = rel_pos_h[coords].transpose(2, 0, 1)  # (64, 32, 32)
    rwT = rel_pos_w[coords].transpose(2, 0, 1)
    rhbd = np.zeros((128, 32, 64), np.float32)
    rwbd = np.zeros((128, 32, 64), np.float32)
    rhbd[0:64, :, 0:32] = rhT
    rhbd[64:128, :, 32:64] = rhT
    rwbd[0:64, :, 0:32] = rwT
    rwbd[64:128, :, 32:64] = rwT
    rhbd = np.ascontiguousarray(rhbd).astype(bf16)
    rwbd = np.ascontiguousarray(rwbd).astype(bf16)

    base_h = np.kron(np.eye(32, dtype=np.float32), np.ones((1, 32), np.float32))
    base_w = np.tile(np.eye(32, dtype=np.float32), (1, 32))
    eyeh = np.ascontiguousarray(
        np.broadcast_to(base_h[:, None, :], (32, 6, 1024))).astype(bf16)
    eyew = np.ascontiguousarray(
        np.broadcast_to(base_w[:, None, :], (32, 6, 1024))).astype(bf16)

    shared = dict(wqkv=wqkv, wproj=wproj, bias3=bias3, wq0=wq0,
                  rhbd=rhbd, rwbd=rwbd, eyeh=eyeh, eyew=eyew)
    in_maps = []
    for b in range(B):
        xT = np.ascontiguousarray(
            x[b].reshape(N, C).T.reshape(6, 128, N).transpose(1, 0, 2)
        ).astype(bf16)
        in_maps.append(dict(xT=xT, **shared))
    return in_maps


_last_results = None


def kernel(x, qkv_w, qkv_b, proj_w, proj_b, rel_pos_h, rel_pos_w):
    global _last_results
    from concourse.bass_utils import run_bass_kernel_spmd

    nc = _get_nc()
    in_maps = _prep_inputs(x, qkv_w, qkv_b, proj_w, proj_b,
                           rel_pos_h, rel_pos_w)
    res = run_bass_kernel_spmd(nc, in_maps, core_ids=list(range(8)))
    _last_results = res
    out = np.stack([
        np.asarray(res.results[b]["out"], np.float32).T.reshape(H, W, C)
        for b in range(B)
    ])
    return out
